# revision 10
# baseline (speedup 1.0000x reference)
"""Trainium2 kernel v3: host-planned compacted pair-gather, 16B bf16 patches.

Per image the host pairs adjacent same-row live pixels (even-aligned) whose
bilinear footprints fit a 2x4 window; each gather descriptor fetches one
16-byte bf16 2x4 patch serving 1-2 output pixels. Slots that hit the
measured indirect-DMA stutter (every 32nd slot of a 16B-element gather)
carry zero-weight dummies. Device: build the 2x4-patch table (8 shifted
casting DVE copies per row-chunk), one big [1,2048,8]bf16 indirect gather
per partition-lane, chunked multiply+reduce combine with streamed bf16
weights (2 results per slot), clipped f32 results stored to DRAM. Host
places results into the zero output.
"""

from contextlib import ExitStack

import numpy as np
import ml_dtypes

import concourse.bass as bass
import concourse.tile as tile
from concourse import mybir
import concourse.bacc as bacc
from concourse import bass_utils

F32 = mybir.dt.float32
BF16 = mybir.dt.bfloat16
I32 = mybir.dt.int32
ALU = mybir.AluOpType

B, H, W = 32, 512, 512
P = 128
NCORES = 8
SLOTS_PER_CORE = 4
NLANE = 768                  # gather slots per lane (known-good SWDGE count)
GOOD_PER_LANE = NLANE - NLANE // 32
IDXCOL_PER_LANE = NLANE // 128

# ---------------------------------------------------------------------------
# Host planner
# ---------------------------------------------------------------------------

def _image_tables(theta_row):
    t = theta_row
    x = np.linspace(-1.0, 1.0, W, dtype=np.float32)
    y = np.linspace(-1.0, 1.0, H, dtype=np.float32)
    xg, yg = np.meshgrid(x, y)
    sx = (t[0] * xg + t[1] * yg + t[2]).astype(np.float32) * np.float32(255.5) + np.float32(255.5)
    sy = (t[3] * xg + t[4] * yg + t[5]).astype(np.float32) * np.float32(255.5) + np.float32(255.5)
    flx = np.floor(sx); fly = np.floor(sy)
    offx = np.array([0., 1., 1., 0.], np.float32)
    offy = np.array([1., 0., 1., 0.], np.float32)
    nx = np.clip(flx[..., None] + offx[None, None, :], 0.0, W - 1.0)
    ny = np.clip(fly[..., None] + offy[None, None, :], 0.0, H - 1.0)
    dx = np.maximum(0.0, 1.0 - np.abs(sx[..., None] - nx)).astype(np.float32)
    dy = np.maximum(0.0, 1.0 - np.abs(sy[..., None] - ny)).astype(np.float32)
    coef = dx * dy
    return ny.astype(np.int32), nx.astype(np.int32), coef, coef.sum(axis=-1) > 0


def _plan_image(theta_row, axis):
    """Pair-based patch stream for one image.

    Returns dict(idxs [npatch] i32, w [npatch, 2, 8] f32, lin [npatch, 2] i64).
    Patch = rows baseR..+1 x cols baseC..+3 window; slot u=1 may be dummy.
    """
    rows, cols, coef, live = _image_tables(theta_row)
    if axis == 1:
        rows = rows.transpose(1, 0, 2)
        cols = cols.transpose(1, 0, 2)
        coef = coef.transpose(1, 0, 2)
        live = live.T
    ii, jj = np.meshgrid(np.arange(H), np.arange(W), indexing="ij")
    lin_grid = (jj * W + ii) if axis == 1 else (ii * W + jj)

    rmin = rows.min(-1); rmax = rows.max(-1)
    cmin = cols.min(-1); cmax = cols.max(-1)
    # even-aligned pair candidates
    pair = np.zeros((H, W), bool)
    pair[:, 0::2] = (live[:, 0::2] & live[:, 1::2]
                     & (np.maximum(rmax[:, 0::2], rmax[:, 1::2])
                        - np.minimum(rmin[:, 0::2], rmin[:, 1::2]) <= 1)
                     & (np.maximum(cmax[:, 0::2], cmax[:, 1::2])
                        - np.minimum(cmin[:, 0::2], cmin[:, 1::2]) <= 3))
    in_pair = np.zeros((H, W), bool)
    in_pair[:, 0::2] = pair[:, 0::2]
    in_pair[:, 1::2] = pair[:, 0::2]
    single = live & ~in_pair

    def build(maskA, maskB=None):
        """Patches for pixels at maskA (u=0) and optionally maskB (u=1)."""
        ia, ja = np.nonzero(maskA)
        npatch = ia.size
        if npatch == 0:
            return (np.zeros(0, np.int32), np.zeros((0, 2, 8), np.float32),
                    np.full((0, 2), -1, np.int64))
        if maskB is not None:
            ib, jb = ia, ja + 1
            bR = np.minimum(np.minimum(rmin[ia, ja], rmin[ib, jb]), H - 2)
            bC = np.minimum(np.minimum(cmin[ia, ja], cmin[ib, jb]), W - 4)
        else:
            bR = np.minimum(rmin[ia, ja], H - 2)
            bC = np.minimum(cmin[ia, ja], W - 4)
        w = np.zeros((npatch, 2, 8), np.float32)
        lin = np.full((npatch, 2), -1, np.int64)
        members = [(0, ia, ja)] + ([(1, ib, jb)] if maskB is not None else [])
        for u, iu, ju in members:
            a = rows[iu, ju] - bR[:, None]
            bb = cols[iu, ju] - bC[:, None]
            assert a.min() >= 0 and a.max() <= 1, (a.min(), a.max())
            assert bb.min() >= 0 and bb.max() <= 3
            v = a * 4 + bb
            np.add.at(w, (np.arange(npatch)[:, None],
                          np.full((npatch, 4), u),
                          v), coef[iu, ju])
            lin[:, u] = lin_grid[iu, ju]
        return (bR * W + bC).astype(np.int32), w, lin

    iP, wP, lP = build(pair, True)
    iS, wS, lS = build(single, None)
    return dict(idxs=np.concatenate([iP, iS]),
                w=np.concatenate([wP, wS]),
                lin=np.concatenate([lP, lS]))


def _choose_axis(theta_row):
    t = theta_row
    mj = max(abs(float(t[0])), abs(float(t[3])))
    mi = max(abs(float(t[1])), abs(float(t[4])))
    return 0 if mj <= mi else 1


def plan_all(theta):
    theta = np.asarray(theta, np.float32)
    nimg = theta.shape[0]
    infos = []
    for bimg in range(nimg):
        pl = _plan_image(theta[bimg], _choose_axis(theta[bimg]))
        infos.append(dict(pl=pl, b=bimg, npatch=pl["idxs"].shape[0]))

    # assignment: sort by npatch desc, slot = pos//8, core by load
    order = sorted(range(nimg), key=lambda i: -infos[i]["npatch"])
    assignments = [[None] * SLOTS_PER_CORE for _ in range(NCORES)]
    slot_lanes = [0] * SLOTS_PER_CORE
    loads = [0] * NCORES
    for pos, bi in enumerate(order):
        m = min(pos // NCORES, SLOTS_PER_CORE - 1)
        c = int(np.argmin([loads[c2] if assignments[c2][m] is None else 1 << 60
                           for c2 in range(NCORES)]))
        assignments[c][m] = bi
        loads[c] += infos[bi]["npatch"]
        lanes = (infos[bi]["npatch"] + GOOD_PER_LANE - 1) // GOOD_PER_LANE
        slot_lanes[m] = max(slot_lanes[m], lanes)

    maxres = 2 * NLANE
    # 32-aligned lane packing into (round, partition)
    lane_slot = []
    for m in range(SLOTS_PER_CORE):
        lane_slot += [m] * slot_lanes[m]
    lane_rp = []
    segments = []
    r, p = 0, 0
    for m in range(SLOTS_PER_CORE):
        left = slot_lanes[m]
        lane_lo = len(lane_rp)
        while left > 0:
            p = ((p + 31) // 32) * 32
            if p >= P:
                r += 1; p = 0
            take = min(left, 32, P - p)
            segments.append((r, m, p, p + take, lane_lo))
            for q in range(take):
                lane_rp.append((r, p + q))
            lane_lo += take
            p += take
            left -= take
    rounds = r + 1
    sig = (NLANE, tuple(slot_lanes), rounds)

    good_slots = np.array([s for s in range(NLANE) if s % 32 != 0], np.int64)
    idxcols_n = sum(slot_lanes) * IDXCOL_PER_LANE + 1
    lane_woff = [l * NLANE * 16 for l in range(len(lane_slot))]
    wtot = len(lane_slot) * NLANE * 16

    lane_base = np.cumsum([0] + list(slot_lanes)).tolist()
    per_core = []
    plins, pposs = [], []
    for c in range(NCORES):
        idxcols = np.zeros((P, idxcols_n), np.int32)
        wts = np.zeros(wtot, np.float32)
        lin_all, pos_all = [], []
        for m in range(SLOTS_PER_CORE):
            bi = assignments[c][m]
            if bi is None:
                continue
            pl = infos[bi]["pl"]
            npatch = pl["idxs"].shape[0]
            nlanes = (npatch + GOOD_PER_LANE - 1) // GOOD_PER_LANE
            for ll in range(nlanes):
                l_glob = lane_base[m] + ll
                base = l_glob * IDXCOL_PER_LANE
                p0 = ll * GOOD_PER_LANE
                p1 = min(npatch, p0 + GOOD_PER_LANE)
                cnt = p1 - p0
                slots = good_slots[:cnt]
                idxcols[slots % 128, base + slots // 128] = pl["idxs"][p0:p1]
                woff = lane_woff[l_glob]
                wlane = np.zeros((NLANE, 2, 8), np.float32)
                wlane[slots] = pl["w"][p0:p1]
                wts[woff:woff + NLANE * 16] = wlane.ravel()
                rr, pp = lane_rp[l_glob]
                lin = pl["lin"][p0:p1]
                su = slots[:, None] * 2 + np.arange(2)[None, :]
                resflat = pp * (rounds * maxres) + rr * maxres + su
                ok = lin >= 0
                lin_all.append(np.int64(infos[bi]["b"]) * (H * W) + lin[ok])
                pos_all.append(resflat[ok] + np.int64(c) * (P * rounds * maxres))
        per_core.append(dict(idxcols=idxcols,
                             weights=wts,
                             imgslots=[assignments[c][m]
                                       for m in range(SLOTS_PER_CORE)]))
        plins.append(np.concatenate(lin_all) if lin_all else np.zeros(0, np.int64))
        pposs.append(np.concatenate(pos_all) if pos_all else np.zeros(0, np.int64))

    placement = dict(lin=np.concatenate(plins), pos=np.concatenate(pposs))
    meta = dict(sig=sig, lane_slot=lane_slot, lane_woff=lane_woff,
                slot_lanes=slot_lanes, rounds=rounds, maxres=maxres,
                wtot=wtot, idxcols_n=idxcols_n, assignments=assignments,
                lane_rp=lane_rp, segments=segments)
    return meta, per_core, placement


# ---------------------------------------------------------------------------
# Device program
# ---------------------------------------------------------------------------

def build_program(nc: bass.Bass, meta):
    slot_lanes = meta["slot_lanes"]
    rounds = meta["rounds"]
    maxres = meta["maxres"]
    lane_slot = meta["lane_slot"]
    idxcols_n = meta["idxcols_n"]
    wtot = meta["wtot"]
    lane_woff = meta["lane_woff"]
    RESCAP = rounds * maxres

    img_d = nc.dram_tensor("image", [SLOTS_PER_CORE, H * W], F32,
                           kind="ExternalInput")
    idx_d = nc.dram_tensor("idxcols", [P, idxcols_n], I32, kind="ExternalInput")
    w_d = nc.dram_tensor("weights", [max(wtot, 16)], F32, kind="ExternalInput")
    res_d = nc.dram_tensor("res", [P, RESCAP], F32, kind="ExternalOutput")
    jp_ds = [nc.dram_tensor(f"jp_{m}", [H * W, 8], BF16, kind="Internal")
             for m in range(SLOTS_PER_CORE)]

    bound_rv = nc.gpsimd.to_reg(H * W - 1)
    IFREE = 2048
    IM = IFREE + 512 + 16

    active = [m for m in range(SLOTS_PER_CORE) if slot_lanes[m] > 0]
    lane_rp = meta["lane_rp"]
    segments = meta["segments"]
    lane_of = {}
    for l, (r, p) in enumerate(lane_rp):
        lane_of[(r, p)] = (l, lane_slot[l])

    with tile.TileContext(nc) as tc, ExitStack() as ctx:
        ip = ctx.enter_context(tc.tile_pool(name="ip", bufs=2))
        jp = ctx.enter_context(tc.tile_pool(name="jp", bufs=2))
        sp = ctx.enter_context(tc.tile_pool(name="sp", bufs=1))
        gp = ctx.enter_context(tc.tile_pool(name="gp", bufs=1))
        wp = ctx.enter_context(tc.tile_pool(name="wp", bufs=2))
        cp = ctx.enter_context(tc.tile_pool(name="cp", bufs=2))

        idxt = sp.tile([P, idxcols_n], I32)
        nc.sync.dma_start(out=idxt[:], in_=idx_d[:])

        # ---- patch-table builds (2x4 windows, bf16) ----
        for m in active:
            i5 = ip.tile([P, IM], F32, tag="i5")
            nc.vector.memset(i5[:, IFREE:], 0.0)
            nc.sync.dma_start(
                out=i5[:, 0:IFREE],
                in_=bass.AP(tensor=img_d, offset=img_d[m].offset,
                            ap=[[IFREE, P], [1, IFREE]]))
            nc.sync.dma_start(
                out=i5[0:127, IFREE:IFREE + 512],
                in_=bass.AP(tensor=img_d, offset=img_d[m].offset + IFREE,
                            ap=[[IFREE, 127], [1, 512]]))
            for cc in range(4):
                for half in range(2):
                    jt = jp.tile([P, 256, 8], BF16, tag="jt")
                    for v in range(8):
                        a, bb = v // 4, v % 4
                        src = i5[:, (cc + a) * 512 + half * 256 + bb:
                                 (cc + a) * 512 + half * 256 + bb + 256]
                        if v % 2 == 0:
                            nc.vector.tensor_copy(jt[:, :, v], src)
                        else:
                            nc.scalar.copy(jt[:, :, v], src)
                    nc.sync.dma_start(
                        out=bass.AP(tensor=jp_ds[m],
                                    offset=(cc * 512 + half * 256) * 8,
                                    ap=[[2048 * 8, P], [1, 256 * 8]]),
                        in_=jt[:])

        # phase fence: gpsimd executes in order; these tiny copies wait on
        # all table builds, so the gathers below run with an idle memory bus
        for m in active:
            ftile = sp.tile([1, 8], F32, tag=f"fence{m}")
            nc.gpsimd.dma_start(out=ftile[:], in_=jp_ds[m][0:1, :])

        # ---- gathers: 4 dst tiles so the DMAs pipeline ----
        patches = []
        for r in range(rounds):
            pts = []
            for g in range(4):
                ptile = gp.tile([P, NLANE, 8], BF16, tag=f"patch{g}")
                pts.append(ptile)
            patches.append(pts)
            for pp in range(32):
                for g in range(4):
                    p = g * 32 + pp
                    ent = lane_of.get((r, p))
                    if ent is None:
                        continue
                    l, m = ent
                    base = l * IDXCOL_PER_LANE
                    nc.gpsimd.indirect_dma_start(
                        out=pts[g][p:p + 1, :, :], out_offset=None,
                        in_=jp_ds[m][:],
                        in_offset=bass.IndirectOffsetOnAxis(
                            ap=idxt[:, base:base + IDXCOL_PER_LANE], axis=0),
                        bounds_check=bound_rv, oob_is_err=False)

        # ---- combine ----
        CH = 256
        nch = NLANE // CH
        for (r, m, pl0, pl1, lane_lo) in segments:
            dl1 = min(P, pl0 + ((pl1 - pl0 + 31) // 32) * 32)
            for ch in range(nch):
                s0 = ch * CH
                ns = CH
                wch = wp.tile([P, CH * 16], BF16, tag="wch")
                woff0 = lane_woff[lane_lo] + s0 * 16
                nc.gpsimd.dma_start(
                    out=wch[pl0:pl1, :],
                    in_=bass.AP(tensor=w_d, offset=woff0,
                                ap=[[NLANE * 16, pl1 - pl0], [1, ns * 16]]))
                prod = cp.tile([P, CH, 16], F32, tag="prod")
                pat = patches[r][pl0 // 32]
                src = bass.AP(
                    tensor=pat.tensor,
                    offset=pat[pl0:dl1, s0:s0 + ns, :].offset,
                    ap=[pat[pl0:dl1, s0:s0 + ns, :].ap[0],
                        [8, ns], [0, 2], [1, 8]])
                wap = bass.AP(
                    tensor=wch.tensor,
                    offset=wch[pl0:dl1, :].offset,
                    ap=[wch[pl0:dl1, :].ap[0], [16, ns], [8, 2], [1, 8]])
                pr = bass.AP(
                    tensor=prod.tensor,
                    offset=prod[pl0:dl1, :, :].offset,
                    ap=[prod[pl0:dl1, :, :].ap[0], [16, ns], [8, 2], [1, 8]])
                nc.vector.tensor_tensor(pr, src, wap, ALU.mult)
                resc = cp.tile([P, CH * 2], F32, tag="resc")
                rap = bass.AP(
                    tensor=resc.tensor,
                    offset=resc[pl0:dl1, :].offset,
                    ap=[resc[pl0:dl1, :].ap[0], [2, ns], [1, 2]])
                nc.vector.tensor_reduce(rap, pr, mybir.AxisListType.X, ALU.add)
                nc.vector.tensor_scalar(resc[pl0:dl1, :], resc[pl0:dl1, :],
                                        0.0, 1.0, ALU.max, ALU.min)
                nc.sync.dma_start(
                    out=bass.AP(tensor=res_d,
                                offset=pl0 * RESCAP + r * maxres + s0 * 2,
                                ap=[[RESCAP, pl1 - pl0], [1, ns * 2]]),
                    in_=resc[pl0:pl1, :])
    return nc


# ---------------------------------------------------------------------------
# Orchestration
# ---------------------------------------------------------------------------

_CACHE = {}


def _get_compiled(meta):
    key = meta["sig"]
    if key not in _CACHE:
        nc = bacc.Bacc("TRN2", target_bir_lowering=False, debug=False,
                       enable_asserts=False)
        build_program(nc, meta)
        nc.compile()
        _CACHE[key] = nc
    return _CACHE[key]


def prepare_run(theta: np.ndarray, image: np.ndarray):
    theta = np.ascontiguousarray(np.asarray(theta, dtype=np.float32))
    image = np.asarray(image, dtype=np.float32).reshape(B, H * W)
    meta, per_core, placement = plan_all(theta)
    nc = _get_compiled(meta)
    in_maps = []
    for c in range(NCORES):
        pc = per_core[c]
        img = np.zeros((SLOTS_PER_CORE, H * W), np.float32)
        for m, bi in enumerate(pc["imgslots"]):
            if bi is not None:
                img[m] = image[bi]
        wts = pc["weights"]
        if wts.size < 16:
            wts = np.zeros(16, np.float32)
        in_maps.append({"image": img, "idxcols": pc["idxcols"],
                        "weights": wts})
    return nc, in_maps, (meta, placement)


def kernel(theta: np.ndarray, image: np.ndarray) -> np.ndarray:
    nc, in_maps, (meta, placement) = prepare_run(theta, image)
    res = bass_utils.run_bass_kernel_spmd(nc, in_maps,
                                          core_ids=list(range(NCORES)))
    resall = np.concatenate([np.asarray(r["res"], np.float32).ravel()
                             for r in res.results])
    out = np.zeros(B * H * W, np.float32)
    out[placement["lin"]] = resall[placement["pos"]]
    return out.reshape(B, H, W, 1)


# revision 13
# speedup vs baseline: 2.5578x; 2.5578x over previous
"""Self-contained Trainium2 Bass kernel for the affine-transformation
(spatial-transformer bilinear resampling) problem.

kernel(theta, image): theta [32,6] f32, image [32,512,512,1] f32
-> [32,512,512,1] f32.  Pure data-parallel: 4 images per NeuronCore, 8 cores.

Per image: build a J4 patch table (J4[r*512+c] = the pixel's 2x2 bilinear
footprint as one contiguous 16-byte row) in DRAM scratch; compute source
coords, clamped patch coords, exact clipped-neighbor weights, and gather
indices on DVE+ACT (the gather-index table is computed twice: once in the
natural layout for weights/repairs and once in the transposed layout the
DMA index reader expects, with bit-identical arithmetic); bulk-gather patch
rows with per-partition-slice indirect DMAs (512 quads per instruction,
four per-chunk destination tiles so the DMAs pipeline instead of
serializing); re-gather the small set of slots the hardware index reader
reads with duplicated/quirked positions; weighted 4-dot combine; clip.

HW indirect-DMA index-stream contract (measured on trn2): a dst [1, N, 4]
slice of a [128, N, 4] SBUF tile consumes N indices; stream slot k reads
idx position (k % 128, base + k // 128) of the [128, N/128] slice, except
k ≡ 0 (mod 32) (reads (q-1, m+1)), k ≡ 127 (mod 128) (offset-dependent
absolute-column quirk), and slot 0 at some slice offsets.  All such slots
are re-gathered canonically; out-of-bounds pixels carry 2^25-tagged indices
dropped by the DMA bounds check.
"""

from contextlib import ExitStack

import numpy as np

import concourse.bass as bass
import concourse.tile as tile
from concourse import mybir

F32 = mybir.dt.float32
I32 = mybir.dt.int32
ALU = mybir.AluOpType
ACTF = mybir.ActivationFunctionType

H = W = 512
P = 128
FREE = 2048          # pixels per partition per image
NCHUNK = 4
CW = 512
M = FREE // P        # 16 idx columns per bulk slice
BIGTAG = 33554432.0  # 2^25 OOB index tag
BOUND = 510 * 512 + 510  # max valid J4 row


def build_kernel(nc: bass.Bass, imgs: int):
    theta_d = nc.dram_tensor("theta", [imgs, 6], F32, kind="ExternalInput")
    img_d = nc.dram_tensor("image", [imgs, H * W], F32, kind="ExternalInput")
    out_d = nc.dram_tensor("out", [imgs, H * W], F32, kind="ExternalOutput")
    j4_ds = [nc.dram_tensor(f"j4scratch{b}", [H * W, 4], F32, kind="Internal")
             for b in range(imgs)]

    for cval in (2.0, 1.0):
        if (F32, cval) not in nc.const_aps.aps:
            t = nc.alloc_sbuf_tensor(f"const-f32-{cval}", [128, 1], F32)
            nc.gpsimd.memset(t.ap(), cval)
            nc.const_aps.aps[(F32, cval)] = t.ap()
    nc.all_engine_barrier()
    bound_rv = nc.gpsimd.to_reg(BOUND)

    with tile.TileContext(nc) as tc, ExitStack() as ctx:
        singles = ctx.enter_context(tc.tile_pool(name="singles", bufs=1))
        imgpool = ctx.enter_context(tc.tile_pool(name="imgpool", bufs=1))
        j4pool = ctx.enter_context(tc.tile_pool(name="j4pool", bufs=1))
        arith = ctx.enter_context(tc.tile_pool(name="arith", bufs=2))
        tpool = ctx.enter_context(tc.tile_pool(name="tpool", bufs=2))
        gixtp = ctx.enter_context(tc.tile_pool(name="gixtp", bufs=2))
        gpool = ctx.enter_context(tc.tile_pool(name="gpool", bufs=2))
        prodp = ctx.enter_context(tc.tile_pool(name="prodp", bufs=1, space="PSUM"))
        opool = ctx.enter_context(tc.tile_pool(name="opool", bufs=2))

        # --- iotas, normal layout: col index [P, CW]; partition index [P,1] ---
        iota_j = singles.tile([P, CW], F32)
        nc.gpsimd.iota(iota_j[:], pattern=[[1, CW]],
                       base=0, channel_multiplier=0,
                       allow_small_or_imprecise_dtypes=True)
        iota_i = singles.tile([P, NCHUNK, CW], F32)
        nc.gpsimd.iota(iota_i[:], pattern=[[P, NCHUNK], [0, CW]],
                       base=0, channel_multiplier=1,
                       allow_small_or_imprecise_dtypes=True)
        # --- iotas, transposed (bulk-gather) layout:
        # position (q, t, m) -> pixel (t, n=128m+q): j = 128*(m%4)+q,
        # i = t + 128*(m//4) ---
        iotaT_j = singles.tile([P, P, M], F32)
        nc.gpsimd.iota(iotaT_j[:], pattern=[[0, P], [0, 4], [P, 4]],
                       base=0, channel_multiplier=1,
                       allow_small_or_imprecise_dtypes=True)
        iotaT_i = singles.tile([P, P, M], F32)
        nc.gpsimd.iota(iotaT_i[:], pattern=[[1, P], [P, 4], [0, 4]],
                       base=0, channel_multiplier=0,
                       allow_small_or_imprecise_dtypes=True)

        for b in range(imgs):
            # ---- theta-derived per-partition scalars ----
            th = arith.tile([P, 6], F32, tag="theta")
            nc.sync.dma_start(
                out=th[:],
                in_=bass.AP(tensor=theta_d, offset=theta_d[b].offset,
                            ap=[[0, P]] + theta_d[b].ap),
            )
            gx = arith.tile([P, 1], F32, tag="gx")
            gy = arith.tile([P, 1], F32, tag="gy")
            tmp0 = arith.tile([P, 1], F32, tag="gtmp")
            nc.vector.tensor_tensor(tmp0[:], th[:, 0:1], th[:, 1:2], ALU.add)
            nc.vector.tensor_tensor(gx[:], th[:, 2:3], tmp0[:], ALU.subtract)
            nc.vector.tensor_scalar(gx[:], gx[:], 255.5, 255.5, ALU.mult, ALU.add)
            nc.vector.tensor_tensor(tmp0[:], th[:, 3:4], th[:, 4:5], ALU.add)
            nc.vector.tensor_tensor(gy[:], th[:, 5:6], tmp0[:], ALU.subtract)
            nc.vector.tensor_scalar(gy[:], gy[:], 255.5, 255.5, ALU.mult, ALU.add)

            # ---- image load ----
            i5 = imgpool.tile([P, FREE + W + 4], F32, tag="i5")
            nc.vector.memset(i5[:, FREE:], 0.0)
            nc.sync.dma_start(
                out=i5[:, 0:FREE],
                in_=bass.AP(tensor=img_d, offset=img_d[b].offset,
                            ap=[[FREE, P], [1, FREE]]),
            )
            nc.sync.dma_start(
                out=i5[0:127, FREE:FREE + W],
                in_=bass.AP(tensor=img_d, offset=img_d[b].offset + FREE,
                            ap=[[FREE, 127], [1, W]]),
            )

            # ---- J4 build + store ----
            for c in range(NCHUNK):
                j4c = j4pool.tile([P, CW, 4], F32, tag="j4c")
                lo = c * CW
                nc.vector.tensor_copy(j4c[:, :, 0], i5[:, lo:lo + CW])
                nc.vector.tensor_copy(j4c[:, :, 1], i5[:, lo + W:lo + W + CW])
                nc.scalar.copy(j4c[:, :, 2], i5[:, lo + 1:lo + 1 + CW])
                nc.scalar.copy(j4c[:, :, 3], i5[:, lo + W + 1:lo + W + 1 + CW])
                nc.sync.dma_start(
                    out=bass.AP(tensor=j4_ds[b], offset=lo * 4,
                                ap=[[FREE * 4, P], [1, CW * 4]]),
                    in_=j4c[:],
                )

            # ---- transposed index pipeline -> gixT [P, 2048+16] int32 ----
            gixT = gixtp.tile([P, P * M + M], I32, tag="gixT")
            nc.vector.memset(gixT[:, P * M:], 0)
            for u in range(4):  # slices of 32 t-values = [P, 512]
                sl = slice(u * 512, (u + 1) * 512)
                ijT = iotaT_j[:].rearrange("p a b -> p (a b)")[:, sl]
                iiT = iotaT_i[:].rearrange("p a b -> p (a b)")[:, sl]

                def coordT(theta_a, theta_b, gamma, cbtag, otag):
                    s = tpool.tile([P, 512], F32, tag="sT")
                    nc.vector.tensor_scalar(s[:], ijT, th[:, theta_a:theta_a + 1],
                                            None, ALU.mult)
                    nc.vector.scalar_tensor_tensor(
                        s[:], iiT, th[:, theta_b:theta_b + 1], s[:],
                        ALU.mult, ALU.add)
                    nc.vector.tensor_scalar(s[:], s[:], gamma[:, 0:1], -2.0,
                                            ALU.add, ALU.max)
                    nc.vector.tensor_scalar(s[:], s[:], 514.0, None, ALU.min)
                    f = tpool.tile([P, 512], F32, tag="fT")
                    nc.vector.tensor_scalar(f[:], s[:], 8388608.0, 8388608.0,
                                            ALU.add, ALU.subtract)
                    fixm = tpool.tile([P, 512], F32, tag="tmpT")
                    nc.vector.tensor_tensor(fixm[:], f[:], s[:], ALU.is_gt)
                    nc.vector.tensor_tensor(f[:], f[:], fixm[:], ALU.subtract)
                    cb = tpool.tile([P, 512], F32, tag=cbtag)
                    nc.vector.tensor_scalar(cb[:], f[:], 0.0, 510.0,
                                            ALU.max, ALU.min)
                    # oob contribution: (s <= -1) + (s >= 512)
                    o1 = tpool.tile([P, 512], F32, tag=otag)
                    nc.vector.tensor_scalar(o1[:], s[:], -1.0, None, ALU.is_le)
                    o2 = tpool.tile([P, 512], F32, tag="tmpT")
                    nc.vector.tensor_scalar(o2[:], s[:], 512.0, None, ALU.is_ge)
                    nc.vector.tensor_tensor(o1[:], o1[:], o2[:], ALU.add)
                    return cb, o1

                cbxT, oxT = coordT(0, 1, gx, "cbxT", "oxT")
                cbyT, oyT = coordT(3, 4, gy, "cbyT", "oyT")
                gf = tpool.tile([P, 512], F32, tag="sT")
                nc.vector.scalar_tensor_tensor(gf[:], cbyT[:], 512.0, cbxT[:],
                                               ALU.mult, ALU.add)
                nc.vector.tensor_tensor(oxT[:], oxT[:], oyT[:], ALU.add)
                nc.vector.scalar_tensor_tensor(gf[:], oxT[:], BIGTAG, gf[:],
                                               ALU.mult, ALU.add)
                nc.vector.tensor_copy(gixT[:, sl], gf[:])

            # ---- bulk gathers: 4 per-chunk tiles, interleaved issue ----
            qts = []
            for g in range(NCHUNK):
                qt = gpool.tile([P, CW, 4], F32, tag=f"quads{g}")
                nc.scalar.memzero(qt[:])
                qts.append(qt)
            # issue order: same-tile successors 4 partitions apart so their
            # descriptors land on different SDMA engines (port swizzle groups
            # partitions {4e..4e+3, 4e+32..}) and retire in parallel
            for t in [4 * a + r for r in range(4) for a in range(32)]:
                for g in range(NCHUNK):
                    nc.gpsimd.indirect_dma_start(
                        out=qts[g][t:t + 1, :, :],
                        out_offset=None,
                        in_=j4_ds[b][:],
                        in_offset=bass.IndirectOffsetOnAxis(
                            ap=gixT[:, t * M + 4 * g:t * M + 4 * g + 4], axis=0),
                        bounds_check=bound_rv,
                        oob_is_err=False,
                    )

            # ---- normal (weights) pipeline per chunk + repairs + combine ----
            for c in range(NCHUNK):
                ii = iota_i[:, c, :]

                def axis_weights(theta_a, theta_b, gamma, cbtag, wptag):
                    s = arith.tile([P, CW], F32, tag="s")
                    nc.vector.tensor_scalar(s[:], iota_j[:],
                                            th[:, theta_a:theta_a + 1],
                                            None, ALU.mult)
                    nc.vector.scalar_tensor_tensor(
                        s[:], ii, th[:, theta_b:theta_b + 1], s[:],
                        ALU.mult, ALU.add)
                    nc.vector.tensor_scalar(s[:], s[:], gamma[:, 0:1], -2.0,
                                            ALU.add, ALU.max)
                    nc.vector.tensor_scalar(s[:], s[:], 514.0, None, ALU.min)
                    f = arith.tile([P, CW], F32, tag="f")
                    nc.vector.tensor_scalar(f[:], s[:], 8388608.0, 8388608.0,
                                            ALU.add, ALU.subtract)
                    fixm = arith.tile([P, CW], F32, tag="tmp")
                    nc.vector.tensor_tensor(fixm[:], f[:], s[:], ALU.is_gt)
                    nc.vector.tensor_tensor(f[:], f[:], fixm[:], ALU.subtract)
                    cb = arith.tile([P, CW], F32, tag=cbtag)
                    nc.vector.tensor_scalar(cb[:], f[:], 0.0, 510.0,
                                            ALU.max, ALU.min)
                    a = arith.tile([P, CW], F32, tag="a")
                    nc.vector.tensor_tensor(a[:], s[:], cb[:], ALU.subtract)
                    wp = arith.tile([P, CW, 2], F32, tag=wptag)
                    m = arith.tile([P, CW], mybir.dt.uint8, tag="m")
                    nc.vector.tensor_scalar(m[:], a[:], 0.0, None, ALU.is_lt)
                    tA = arith.tile([P, CW], F32, tag="tA")
                    nc.scalar.activation(tA[:], a[:], ACTF.Relu, bias=1.0,
                                         scale=-1.0)
                    nc.vector.tensor_copy(wp[:, :, 0], tA[:])
                    tA = arith.tile([P, CW], F32, tag="tA")
                    nc.scalar.activation(tA[:], a[:], ACTF.Relu, bias=2.0,
                                         scale=2.0)
                    nc.vector.copy_predicated(wp[:, :, 0], m[:], tA[:])
                    m = arith.tile([P, CW], mybir.dt.uint8, tag="m")
                    nc.vector.tensor_scalar(m[:], a[:], 1.0, None, ALU.is_ge)
                    tA = arith.tile([P, CW], F32, tag="tA")
                    nc.scalar.activation(tA[:], a[:], ACTF.Relu)
                    nc.vector.tensor_copy(wp[:, :, 1], tA[:])
                    tA = arith.tile([P, CW], F32, tag="tA")
                    nc.scalar.activation(tA[:], a[:], ACTF.Relu, bias=2.0,
                                         scale=-1.0)
                    nc.vector.tensor_scalar(tA[:], tA[:], 2.0, None, ALU.mult)
                    nc.vector.copy_predicated(wp[:, :, 1], m[:], tA[:])
                    return cb, wp

                cbx, wpx = axis_weights(0, 1, gx, "cbx", "wpx")
                cby, wpy = axis_weights(3, 4, gy, "cby", "wpy")

                gixf = arith.tile([P, CW], F32, tag="f")
                nc.vector.scalar_tensor_tensor(gixf[:], cby[:], 512.0, cbx[:],
                                               ALU.mult, ALU.add)
                gix = arith.tile([P, CW], I32, tag="gix")
                nc.vector.tensor_copy(gix[:], gixf[:])

                # repairs: stutter slots (k = 32j) + k = 127 mod 128 quirk
                rep = sorted(set([0] + list(range(32, CW, 32)) + [127, 255, 383, 511]))
                for local in rep:
                    nc.gpsimd.indirect_dma_start(
                        out=qts[c][:, local, :],
                        out_offset=None,
                        in_=j4_ds[b][:],
                        in_offset=bass.IndirectOffsetOnAxis(
                            ap=gix[:, local:local + 1], axis=0),
                    )

                qc = qts[c][:]
                vy_b = bass.AP(
                    tensor=wpy.tensor,
                    offset=wpy[:].offset,
                    ap=[wpy[:].ap[0], [2, CW], [0, 2], [1, 2]],
                )
                prod = prodp.tile([P, CW, 2, 2], F32, tag="prod")
                nc.vector.tensor_tensor(prod[:], qc, vy_b, ALU.mult)
                rp = prodp.tile([P, CW, 2], F32, tag="rp")
                nc.vector.tensor_reduce(rp[:], prod[:], mybir.AxisListType.X,
                                        ALU.add)
                nc.vector.tensor_tensor(rp[:], rp[:], wpx[:], ALU.mult)
                res = opool.tile([P, CW], F32, tag="res")
                nc.vector.tensor_reduce(res[:], rp[:], mybir.AxisListType.X,
                                        ALU.add)
                nc.vector.tensor_scalar(res[:], res[:], 0.0, 1.0, ALU.max,
                                        ALU.min)
                nc.sync.dma_start(
                    out=bass.AP(tensor=out_d,
                                offset=out_d[b].offset + c * P * W,
                                ap=[[W, P], [1, CW]]),
                    in_=res[:],
                )
    return nc


import concourse.bacc as bacc
from concourse import bass_utils

B = 32
NCORES = 8
IMGS_PER_CORE = B // NCORES

_CACHE = {}


def _get_compiled():
    if "nc" not in _CACHE:
        nc = bacc.Bacc("TRN2", target_bir_lowering=False, debug=False,
                       enable_asserts=False)
        build_kernel(nc, IMGS_PER_CORE)
        nc.compile()
        _CACHE["nc"] = nc
    return _CACHE["nc"]


def _balance(theta):
    """Assign images to cores balancing in-bounds pixel load (DMA retire
    scales with it). Pure host-side sharding; outputs are un-permuted."""
    g = np.linspace(0.0, 511.0, 64, dtype=np.float32)
    J, I = np.meshgrid(g, g)
    loads = []
    for b in range(B):
        t = theta[b]
        sx = t[0] * J + t[1] * I + 255.5 * (t[2] + 1 - t[0] - t[1])
        sy = t[3] * J + t[4] * I + 255.5 * (t[5] + 1 - t[3] - t[4])
        loads.append(float(((sx > -1) & (sx < 512) &
                            (sy > -1) & (sy < 512)).mean()))
    order = np.argsort(loads)[::-1]
    coreload = [0.0] * NCORES
    assign = [[] for _ in range(NCORES)]
    for idx in order:
        k = min((c for c in range(NCORES) if len(assign[c]) < IMGS_PER_CORE),
                key=lambda c: coreload[c])
        assign[k].append(int(idx))
        coreload[k] += loads[idx]
    return [i for c in range(NCORES) for i in assign[c]]


def prepare_run(theta: np.ndarray, image: np.ndarray):
    theta = np.ascontiguousarray(np.asarray(theta, dtype=np.float32))
    image = np.asarray(image, dtype=np.float32)
    img_flat = np.ascontiguousarray(image.reshape(B, H * W))

    perm = _balance(theta)
    nc = _get_compiled()
    in_maps = []
    for k in range(NCORES):
        ids = perm[k * IMGS_PER_CORE:(k + 1) * IMGS_PER_CORE]
        in_maps.append({"theta": np.ascontiguousarray(theta[ids]),
                        "image": np.ascontiguousarray(img_flat[ids])})
    return nc, in_maps, perm


def kernel(theta: np.ndarray, image: np.ndarray) -> np.ndarray:
    nc, in_maps, perm = prepare_run(theta, image)

    res = bass_utils.run_bass_kernel_spmd(nc, in_maps,
                                          core_ids=list(range(NCORES)))
    rows = np.concatenate([r["out"] for r in res.results], axis=0)
    out = np.empty_like(rows)
    for pos, img in enumerate(perm):
        out[img] = rows[pos]
    return out.reshape(B, H, W, 1)

